# revision 5
# baseline (speedup 1.0000x reference)
"""Trainium2 Bass kernel for nn_BasePBC (PBC tap products).

Math:
  Reference computes, for each tap s=(m,n) with |m*n|<=25, |m|,|n|<=25:
      En  = roll(E, n); Emn = roll(E, m+n); Em = roll(E, m)   (roll along W)
      A   = En * conj(Emn);  Asum = A + flip_modes(A);  F = Asum * Em
  Key identities used here:
      roll(E,n)*conj(roll(E,m+n)) = roll(C_m, n) with C_m = E*conj(roll(E,m))
      Asum(mode0) == Asum(mode1) == roll(B_m, n),  B_m = sum_mu C_m[mu]
  So per tap:  F_mu[w] = B_m[w-n] * E_mu[w-m]   -- only 51 distinct B_m.

Distribution (SPMD, 8 cores, identical program):
  Shard W into 8 slices of 2048. Each core computes ALL 449 taps on its
  slice. Per-core differences live purely in the input data (a haloed
  window of E). On-chip layout puts (tap,b) rows on the 128 partitions;
  circular shifts become per-row flat-element offsets realized with
  indirect (gather) DMAs from DRAM using constant offset tables.
"""

import numpy as np

import concourse.bass as bass
import concourse.bacc as bacc
import concourse.mybir as mybir
from concourse.tile import TileContext

# ---------------- problem constants (must match reference.py) --------------
RHO, L = 1.0, 50
TAPS = [
    (m, n)
    for m in range(-L // 2, L // 2 + 1)
    for n in range(-L // 2, L // 2 + 1)
    if abs(m * n) <= RHO * L // 2
]
S = len(TAPS)  # 449
B, W, NMODES = 2, 16384, 2
NCORES = 8
WLOC = W // NCORES  # 2048
EHALO = 64  # halo on each side of the local E window
EW = WLOC + 2 * EHALO  # 2176: e-plane row width
MS = sorted({m for m, _ in TAPS})  # -25..25
NM = len(MS)  # 51
M_IDX = {m: i for i, m in enumerate(MS)}
BMH = 32  # B_m halo (covers |n| <= 25)
BMW = WLOC + 2 * BMH  # 2112
NROWS = S * B  # 898   (row r = t*2 + b)
NB = 8
BR = (NROWS + NB - 1) // NB  # 113 rows per block (last: 107)
NCOLS = 8 + 6 * NB  # offset-table columns

FP = mybir.dt.float16
NPFP = np.float16


def _pidx(b, mu, ri):
    return (b * 2 + mu) * 2 + ri


def _build_offsets() -> np.ndarray:
    offs = np.zeros((128, NCOLS), dtype=np.int32)
    # --- B_m phase tables (cols 0..7), rows (m_idx, b) ---
    # col j in 0..3: unshifted plane (mu, ri); col 4..7: plane shifted by m
    for mi, m in enumerate(MS):
        for b in range(B):
            r = mi * 2 + b
            for j, (mu, ri) in enumerate([(0, 0), (0, 1), (1, 0), (1, 1)]):
                base = _pidx(b, mu, ri) * EW
                offs[r, j] = base + (EHALO - BMH)
                offs[r, 4 + j] = base + (EHALO - BMH) - m
    # --- F phase tables (cols 8 + k*6 + j), rows p in block k ---
    for k in range(NB):
        r0 = k * BR
        for p in range(min(BR, NROWS - r0)):
            r = r0 + p
            t, b = r // 2, r % 2
            m, n = TAPS[t]
            c0 = 8 + k * 6
            bmrow = M_IDX[m] * 2 + b
            offs[p, c0 + 0] = (bmrow * 2 + 0) * BMW + BMH - n  # Ar
            offs[p, c0 + 1] = (bmrow * 2 + 1) * BMW + BMH - n  # Ai
            offs[p, c0 + 2] = _pidx(b, 0, 0) * EW + EHALO - m  # er0
            offs[p, c0 + 3] = _pidx(b, 0, 1) * EW + EHALO - m  # ei0
            offs[p, c0 + 4] = _pidx(b, 1, 0) * EW + EHALO - m  # er1
            offs[p, c0 + 5] = _pidx(b, 1, 1) * EW + EHALO - m  # ei1
    return offs


def _build_nc():
    nc = bacc.Bacc("TRN2", debug=False, target_bir_lowering=False)
    e_dram = nc.dram_tensor("e_planes", [8, EW], FP, kind="ExternalInput")
    offs_dram = nc.dram_tensor("offs", [128, NCOLS], mybir.dt.int32, kind="ExternalInput")
    out_dram = nc.dram_tensor("out", [NROWS, 2, 2, WLOC], FP, kind="ExternalOutput")
    bm_dram = nc.dram_tensor("bm_scratch", [NM * B, 2, BMW], FP)  # Internal scratch

    NMB = NM * B  # 102
    with TileContext(nc) as tc:
        with tc.tile_pool(name="const", bufs=1) as cpool:
            offs = cpool.tile([128, NCOLS], mybir.dt.int32)
            nc.sync.dma_start(out=offs[:], in_=offs_dram[:])

            # ---------------- B_m phase ----------------
            with tc.tile_pool(name="bmph", bufs=1) as bpool:
                g = []
                for j in range(8):
                    t_ = bpool.tile([NMB, BMW], FP, tag=f"bmop{j}", name=f"bmop{j}")
                    nc.gpsimd.indirect_dma_start(
                        out=t_[:],
                        out_offset=None,
                        in_=e_dram[:],
                        in_offset=bass.IndirectOffsetOnAxis(
                            ap=offs[:NMB, j : j + 1], axis=1
                        ),
                    )
                    g.append(t_)
                ur0, ui0, ur1, ui1, sr0, si0, sr1, si1 = g
                bm = bpool.tile([NMB, 2, BMW], FP, tag="bm")
                tp = [bpool.tile([NMB, BMW], FP, tag=f"bmt{i}", name=f"bmt{i}") for i in range(4)]
                V = nc.vector
                # real part: sum_mu (ur*sr + ui*si)
                V.tensor_mul(out=tp[0][:], in0=ur0[:], in1=sr0[:])
                V.tensor_mul(out=tp[1][:], in0=ui0[:], in1=si0[:])
                V.tensor_mul(out=tp[2][:], in0=ur1[:], in1=sr1[:])
                V.tensor_mul(out=tp[3][:], in0=ui1[:], in1=si1[:])
                V.tensor_add(out=tp[0][:], in0=tp[0][:], in1=tp[1][:])
                V.tensor_add(out=tp[2][:], in0=tp[2][:], in1=tp[3][:])
                V.tensor_add(out=bm[:, 0, :], in0=tp[0][:], in1=tp[2][:])
                # imag part: sum_mu (ui*sr - ur*si)
                V.tensor_mul(out=tp[0][:], in0=ui0[:], in1=sr0[:])
                V.tensor_mul(out=tp[1][:], in0=ur0[:], in1=si0[:])
                V.tensor_mul(out=tp[2][:], in0=ui1[:], in1=sr1[:])
                V.tensor_mul(out=tp[3][:], in0=ur1[:], in1=si1[:])
                V.tensor_sub(out=tp[0][:], in0=tp[0][:], in1=tp[1][:])
                V.tensor_sub(out=tp[2][:], in0=tp[2][:], in1=tp[3][:])
                V.tensor_add(out=bm[:, 1, :], in0=tp[0][:], in1=tp[2][:])
                nc.sync.dma_start(out=bm_dram[:], in_=bm[:])

            # ---------------- F phase ----------------
            with (
                tc.tile_pool(name="fop", bufs=3) as fpool,
                tc.tile_pool(name="ftmp", bufs=4) as tpool,
                tc.tile_pool(name="fout", bufs=3) as opool,
            ):
                for k in range(NB):
                    r0 = k * BR
                    br = min(BR, NROWS - r0)
                    c0 = 8 + k * 6
                    srcs = [bm_dram, bm_dram, e_dram, e_dram, e_dram, e_dram]
                    g = []
                    for j, src in enumerate(srcs):
                        t_ = fpool.tile([128, WLOC], FP, tag=f"fop{j}", name=f"fop{j}")
                        nc.gpsimd.indirect_dma_start(
                            out=t_[:br],
                            out_offset=None,
                            in_=src[:],
                            in_offset=bass.IndirectOffsetOnAxis(
                                ap=offs[:br, c0 + j : c0 + j + 1],
                                axis=len(src.shape) - 1,
                            ),
                        )
                        g.append(t_)
                    ar, ai, er0, ei0, er1, ei1 = g
                    f = opool.tile([128, 2, 2, WLOC], FP, tag="f")
                    V = nc.vector
                    for mu, (er, ei) in enumerate([(er0, ei0), (er1, ei1)]):
                        p = tpool.tile([128, WLOC], FP, tag="p", name="p")
                        q = tpool.tile([128, WLOC], FP, tag="q", name="q")
                        V.tensor_mul(out=p[:br], in0=ar[:br], in1=er[:br])
                        V.tensor_mul(out=q[:br], in0=ai[:br], in1=ei[:br])
                        V.tensor_sub(out=f[:br, mu, 0, :], in0=p[:br], in1=q[:br])
                        p2 = tpool.tile([128, WLOC], FP, tag="p2", name="p2")
                        q2 = tpool.tile([128, WLOC], FP, tag="q2", name="q2")
                        V.tensor_mul(out=p2[:br], in0=ar[:br], in1=ei[:br])
                        V.tensor_mul(out=q2[:br], in0=ai[:br], in1=er[:br])
                        V.tensor_add(out=f[:br, mu, 1, :], in0=p2[:br], in1=q2[:br])
                    nc.sync.dma_start(out=out_dram[r0 : r0 + br], in_=f[:br])
    nc.compile()
    return nc


# ---------------- host side: cached compiled executable --------------------
_CACHE: dict = {}


def _get_runner():
    """Build nc once and wrap a cached jitted SPMD executor (modeled on
    concourse.bass2jax.run_bass_via_pjrt, but reusable across calls)."""
    if "run" in _CACHE:
        return _CACHE["run"]

    import jax
    from jax.sharding import Mesh, PartitionSpec
    from jax.experimental.shard_map import shard_map
    from concourse import bass2jax

    nc = _build_nc()
    bass2jax.install_neuronx_cc_hook()

    partition_name = nc.partition_id_tensor.name if nc.partition_id_tensor else None
    in_names, out_names, out_avals = [], [], []
    for alloc in nc.m.functions[0].allocations:
        if not isinstance(alloc, mybir.MemoryLocationSet):
            continue
        name = alloc.memorylocations[0].name
        if alloc.kind == "ExternalInput":
            if name != partition_name:
                in_names.append(name)
        elif alloc.kind == "ExternalOutput":
            out_names.append(name)
            out_avals.append(
                jax.core.ShapedArray(tuple(alloc.tensor_shape), mybir.dt.np(alloc.dtype))
            )
    n_params = len(in_names)
    n_outs = len(out_avals)
    all_in_names = list(in_names) + list(out_names)
    if partition_name is not None:
        all_in_names.append(partition_name)
    donate = tuple(range(n_params, n_params + n_outs))

    def _body(*args):
        operands = list(args)
        if partition_name is not None:
            operands.append(bass2jax.partition_id_tensor())
        outs = bass2jax._bass_exec_p.bind(
            *operands,
            out_avals=tuple(out_avals),
            in_names=tuple(all_in_names),
            out_names=tuple(out_names),
            lowering_input_output_aliases=(),
            sim_require_finite=True,
            sim_require_nnan=True,
            nc=nc,
        )
        return tuple(outs)

    devices = jax.devices()[:NCORES]
    assert len(devices) == NCORES
    mesh = Mesh(np.asarray(devices), ("core",))
    in_specs = (PartitionSpec("core"),) * (n_params + n_outs)
    out_specs = (PartitionSpec("core"),) * n_outs
    smapped = shard_map(
        _body, mesh=mesh, in_specs=in_specs, out_specs=out_specs, check_rep=False
    )
    sharded = jax.jit(smapped, donate_argnums=donate, keep_unused=True)
    _CACHE["sharded_nodonate"] = jax.jit(smapped, keep_unused=True)
    _CACHE["in_names"] = in_names
    _CACHE["out_names"] = out_names
    _CACHE["out_avals"] = out_avals

    def run(in_maps, device_only=False):
        concat_in = [
            np.concatenate([np.asarray(in_maps[c][nm]) for c in range(NCORES)], axis=0)
            for nm in in_names
        ]
        concat_zeros = [
            np.zeros((NCORES * av.shape[0], *av.shape[1:]), av.dtype) for av in out_avals
        ]
        out_arrs = sharded(*concat_in, *concat_zeros)
        if device_only:
            for o in out_arrs:
                o.block_until_ready()
            return None
        return [
            {
                nm: np.asarray(out_arrs[i]).reshape(NCORES, *out_avals[i].shape)[c]
                for i, nm in enumerate(out_names)
            }
            for c in range(NCORES)
        ]

    _CACHE["run"] = run
    return run


def _make_in_maps(E_real: np.ndarray, E_imag: np.ndarray):
    offs = _CACHE.get("offs")
    if offs is None:
        offs = _CACHE["offs"] = _build_offsets()
    E_real = np.asarray(E_real, dtype=np.float32)
    E_imag = np.asarray(E_imag, dtype=np.float32)
    in_maps = []
    for c in range(NCORES):
        idx = np.arange(c * WLOC - EHALO, (c + 1) * WLOC + EHALO) % W
        planes = np.empty((8, EW), dtype=NPFP)
        for b in range(B):
            for mu in range(NMODES):
                planes[_pidx(b, mu, 0)] = E_real[b, idx, mu].astype(NPFP)
                planes[_pidx(b, mu, 1)] = E_imag[b, idx, mu].astype(NPFP)
        in_maps.append({"e_planes": planes, "offs": offs})
    return in_maps


def _assemble(results) -> np.ndarray:
    out = np.empty((B, W, NMODES, S), dtype=np.complex64)
    for c in range(NCORES):
        o = results[c]["out"].astype(np.float32).reshape(S, B, 2, 2, WLOC)
        cx = o[:, :, :, 0, :] + 1j * o[:, :, :, 1, :]  # [S, B, mu, WLOC]
        out[:, c * WLOC : (c + 1) * WLOC, :, :] = cx.transpose(1, 3, 2, 0)
    return out


def kernel(E_real: np.ndarray, E_imag: np.ndarray) -> np.ndarray:
    run = _get_runner()
    in_maps = _make_in_maps(E_real, E_imag)
    return _assemble(run(in_maps))


def bench(E_real: np.ndarray, E_imag: np.ndarray, iters: int = 50):
    """Time device execution with device-resident inputs (no donation, no
    host transfers in the loop). Returns (seconds_per_iter, outputs)."""
    import time
    import jax

    run = _get_runner()  # ensures nc built + hook installed
    sharded_nodonate = _CACHE["sharded_nodonate"]
    in_names, out_avals = _CACHE["in_names"], _CACHE["out_avals"]
    in_maps = _make_in_maps(E_real, E_imag)
    concat_in = [
        np.concatenate([np.asarray(in_maps[c][nm]) for c in range(NCORES)], axis=0)
        for nm in in_names
    ]
    concat_zeros = [
        np.zeros((NCORES * av.shape[0], *av.shape[1:]), av.dtype) for av in out_avals
    ]
    args = [jax.device_put(a) for a in (*concat_in, *concat_zeros)]
    out = sharded_nodonate(*args)  # warm-up/compile
    jax.block_until_ready(out)
    # measure marginal cost per iteration at two batch sizes to cancel
    # fixed per-batch overhead
    def loop(n):
        t0 = time.perf_counter()
        outs = [sharded_nodonate(*args) for _ in range(n)]
        jax.block_until_ready(outs)
        return time.perf_counter() - t0

    loop(3)
    t_small = loop(max(2, iters // 10))
    t_big = loop(iters)
    per_iter = (t_big - t_small) / (iters - max(2, iters // 10))
    return per_iter, out


# revision 7
# speedup vs baseline: 14.0654x; 14.0654x over previous
"""Trainium2 Bass kernel for nn_BasePBC (PBC tap products).

Math:
  Reference computes, for each tap s=(m,n) with |m*n|<=25, |m|,|n|<=25:
      En  = roll(E, n); Emn = roll(E, m+n); Em = roll(E, m)   (roll along W)
      A   = En * conj(Emn);  Asum = A + flip_modes(A);  F = Asum * Em
  Key identities used here:
      roll(E,n)*conj(roll(E,m+n)) = roll(C_m, n) with C_m = E*conj(roll(E,m))
      Asum(mode0) == Asum(mode1) == roll(B_m, n),  B_m = sum_mu C_m[mu]
  So per tap:  F_mu[w] = B_m[w-n] * E_mu[w-m]   -- only 51 distinct B_m.

Distribution (SPMD, 8 cores, identical program):
  Shard W into 8 slices of 2048. Each core computes ALL 449 taps on its
  slice. Per-core differences live purely in the input data (a haloed
  window of E). On-chip layout puts (tap,b) rows on the 128 partitions;
  circular shifts become per-row flat-element offsets realized with
  indirect (gather) DMAs from DRAM using constant offset tables.
"""

import numpy as np

import concourse.bass as bass
import concourse.bacc as bacc
import concourse.mybir as mybir
from concourse.tile import TileContext

# ---------------- problem constants (must match reference.py) --------------
RHO, L = 1.0, 50
TAPS = [
    (m, n)
    for m in range(-L // 2, L // 2 + 1)
    for n in range(-L // 2, L // 2 + 1)
    if abs(m * n) <= RHO * L // 2
]
S = len(TAPS)  # 449
B, W, NMODES = 2, 16384, 2
NCORES = 8
WLOC = W // NCORES  # 2048
EHALO = 64  # halo on each side of the local E window
EW = WLOC + 2 * EHALO  # 2176: e-plane row width
MS = sorted({m for m, _ in TAPS})  # -25..25
NM = len(MS)  # 51
M_IDX = {m: i for i, m in enumerate(MS)}
BMH = 32  # B_m halo (covers |n| <= 25)
BMW = WLOC + 2 * BMH  # 2112
NROWS = S * B  # 898   (row r = t*2 + b)
NB = 8
BR = (NROWS + NB - 1) // NB  # 113 rows per block (last: 107)
NCOLS = 8 + 6 * NB  # offset-table columns

FP = mybir.dt.float16
NPFP = np.float16


def _pidx(b, mu, ri):
    return (b * 2 + mu) * 2 + ri


def _build_offsets() -> np.ndarray:
    offs = np.zeros((128, NCOLS), dtype=np.int32)
    # --- B_m phase tables (cols 0..7), rows (m_idx, b) ---
    # col j in 0..3: unshifted plane (mu, ri); col 4..7: plane shifted by m
    for mi, m in enumerate(MS):
        for b in range(B):
            r = mi * 2 + b
            for j, (mu, ri) in enumerate([(0, 0), (0, 1), (1, 0), (1, 1)]):
                base = _pidx(b, mu, ri) * EW
                offs[r, j] = base + (EHALO - BMH)
                offs[r, 4 + j] = base + (EHALO - BMH) - m
    # --- F phase tables (cols 8 + k*6 + j), rows p in block k ---
    for k in range(NB):
        r0 = k * BR
        for p in range(min(BR, NROWS - r0)):
            r = r0 + p
            t, b = r // 2, r % 2
            m, n = TAPS[t]
            c0 = 8 + k * 6
            bmrow = M_IDX[m] * 2 + b
            offs[p, c0 + 0] = (bmrow * 2 + 0) * BMW + BMH - n  # Ar
            offs[p, c0 + 1] = (bmrow * 2 + 1) * BMW + BMH - n  # Ai
            offs[p, c0 + 2] = _pidx(b, 0, 0) * EW + EHALO - m  # er0
            offs[p, c0 + 3] = _pidx(b, 0, 1) * EW + EHALO - m  # ei0
            offs[p, c0 + 4] = _pidx(b, 1, 0) * EW + EHALO - m  # er1
            offs[p, c0 + 5] = _pidx(b, 1, 1) * EW + EHALO - m  # ei1
    return offs


def _build_nc():
    nc = bacc.Bacc("TRN2", debug=False, target_bir_lowering=False)
    e_dram = nc.dram_tensor("e_planes", [8, EW], FP, kind="ExternalInput")
    offs_dram = nc.dram_tensor("offs", [128, NCOLS], mybir.dt.int32, kind="ExternalInput")
    out_dram = nc.dram_tensor("out", [NROWS, 2, 2, WLOC], FP, kind="ExternalOutput")
    bm_dram = nc.dram_tensor("bm_scratch", [NM * B, 2, BMW], FP)  # Internal scratch

    NMB = NM * B  # 102
    with TileContext(nc) as tc:
        with tc.tile_pool(name="const", bufs=1) as cpool:
            offs = cpool.tile([128, NCOLS], mybir.dt.int32)
            nc.sync.dma_start(out=offs[:], in_=offs_dram[:])

            # ---------------- B_m phase ----------------
            with tc.tile_pool(name="bmph", bufs=1) as bpool:
                g = []
                for j in range(8):
                    t_ = bpool.tile([NMB, BMW], FP, tag=f"bmop{j}", name=f"bmop{j}")
                    nc.gpsimd.indirect_dma_start(
                        out=t_[:],
                        out_offset=None,
                        in_=e_dram[:],
                        in_offset=bass.IndirectOffsetOnAxis(
                            ap=offs[:NMB, j : j + 1], axis=1
                        ),
                    )
                    g.append(t_)
                ur0, ui0, ur1, ui1, sr0, si0, sr1, si1 = g
                bm = bpool.tile([NMB, 2, BMW], FP, tag="bm")
                tp = [bpool.tile([NMB, BMW], FP, tag=f"bmt{i}", name=f"bmt{i}") for i in range(4)]
                V = nc.vector
                # real part: sum_mu (ur*sr + ui*si)
                V.tensor_mul(out=tp[0][:], in0=ur0[:], in1=sr0[:])
                V.tensor_mul(out=tp[1][:], in0=ui0[:], in1=si0[:])
                V.tensor_mul(out=tp[2][:], in0=ur1[:], in1=sr1[:])
                V.tensor_mul(out=tp[3][:], in0=ui1[:], in1=si1[:])
                V.tensor_add(out=tp[0][:], in0=tp[0][:], in1=tp[1][:])
                V.tensor_add(out=tp[2][:], in0=tp[2][:], in1=tp[3][:])
                V.tensor_add(out=bm[:, 0, :], in0=tp[0][:], in1=tp[2][:])
                # imag part: sum_mu (ui*sr - ur*si)
                V.tensor_mul(out=tp[0][:], in0=ui0[:], in1=sr0[:])
                V.tensor_mul(out=tp[1][:], in0=ur0[:], in1=si0[:])
                V.tensor_mul(out=tp[2][:], in0=ui1[:], in1=sr1[:])
                V.tensor_mul(out=tp[3][:], in0=ur1[:], in1=si1[:])
                V.tensor_sub(out=tp[0][:], in0=tp[0][:], in1=tp[1][:])
                V.tensor_sub(out=tp[2][:], in0=tp[2][:], in1=tp[3][:])
                V.tensor_add(out=bm[:, 1, :], in0=tp[0][:], in1=tp[2][:])
                nc.sync.dma_start(out=bm_dram[:], in_=bm[:])

            # ---------------- F phase ----------------
            with (
                tc.tile_pool(name="fop", bufs=3) as fpool,
                tc.tile_pool(name="ftmp", bufs=4) as tpool,
                tc.tile_pool(name="fout", bufs=3) as opool,
            ):
                for k in range(NB):
                    r0 = k * BR
                    br = min(BR, NROWS - r0)
                    c0 = 8 + k * 6
                    srcs = [bm_dram, bm_dram, e_dram, e_dram, e_dram, e_dram]
                    g = []
                    for j, src in enumerate(srcs):
                        t_ = fpool.tile([128, WLOC], FP, tag=f"fop{j}", name=f"fop{j}")
                        nc.gpsimd.indirect_dma_start(
                            out=t_[:br],
                            out_offset=None,
                            in_=src[:],
                            in_offset=bass.IndirectOffsetOnAxis(
                                ap=offs[:br, c0 + j : c0 + j + 1],
                                axis=len(src.shape) - 1,
                            ),
                        )
                        g.append(t_)
                    ar, ai, er0, ei0, er1, ei1 = g
                    f = opool.tile([128, 2, 2, WLOC], FP, tag="f")
                    V = nc.vector
                    for mu, (er, ei) in enumerate([(er0, ei0), (er1, ei1)]):
                        p = tpool.tile([128, WLOC], FP, tag="p", name="p")
                        q = tpool.tile([128, WLOC], FP, tag="q", name="q")
                        V.tensor_mul(out=p[:br], in0=ar[:br], in1=er[:br])
                        V.tensor_mul(out=q[:br], in0=ai[:br], in1=ei[:br])
                        V.tensor_sub(out=f[:br, mu, 0, :], in0=p[:br], in1=q[:br])
                        p2 = tpool.tile([128, WLOC], FP, tag="p2", name="p2")
                        q2 = tpool.tile([128, WLOC], FP, tag="q2", name="q2")
                        V.tensor_mul(out=p2[:br], in0=ar[:br], in1=ei[:br])
                        V.tensor_mul(out=q2[:br], in0=ai[:br], in1=er[:br])
                        V.tensor_add(out=f[:br, mu, 1, :], in0=p2[:br], in1=q2[:br])
                    nc.sync.dma_start(out=out_dram[r0 : r0 + br], in_=f[:br])
    nc.compile()
    return nc


# ---------------- host side: cached compiled executable --------------------
_CACHE: dict = {}


def _get_runner():
    """Build nc once and wrap a cached jitted SPMD executor (modeled on
    concourse.bass2jax.run_bass_via_pjrt, but reusable across calls)."""
    if "run" in _CACHE:
        return _CACHE["run"]

    import jax
    from jax.sharding import Mesh, PartitionSpec
    from jax.experimental.shard_map import shard_map
    from concourse import bass2jax

    nc = _build_nc()
    bass2jax.install_neuronx_cc_hook()

    partition_name = nc.partition_id_tensor.name if nc.partition_id_tensor else None
    in_names, out_names, out_avals = [], [], []
    for alloc in nc.m.functions[0].allocations:
        if not isinstance(alloc, mybir.MemoryLocationSet):
            continue
        name = alloc.memorylocations[0].name
        if alloc.kind == "ExternalInput":
            if name != partition_name:
                in_names.append(name)
        elif alloc.kind == "ExternalOutput":
            out_names.append(name)
            out_avals.append(
                jax.core.ShapedArray(tuple(alloc.tensor_shape), mybir.dt.np(alloc.dtype))
            )
    n_params = len(in_names)
    n_outs = len(out_avals)
    all_in_names = list(in_names) + list(out_names)
    if partition_name is not None:
        all_in_names.append(partition_name)
    donate = tuple(range(n_params, n_params + n_outs))

    def _body(*args):
        operands = list(args)
        if partition_name is not None:
            operands.append(bass2jax.partition_id_tensor())
        outs = bass2jax._bass_exec_p.bind(
            *operands,
            out_avals=tuple(out_avals),
            in_names=tuple(all_in_names),
            out_names=tuple(out_names),
            lowering_input_output_aliases=(),
            sim_require_finite=True,
            sim_require_nnan=True,
            nc=nc,
        )
        return tuple(outs)

    devices = jax.devices()[:NCORES]
    assert len(devices) == NCORES
    mesh = Mesh(np.asarray(devices), ("core",))
    in_specs = (PartitionSpec("core"),) * (n_params + n_outs)
    out_specs = (PartitionSpec("core"),) * n_outs
    smapped = shard_map(
        _body, mesh=mesh, in_specs=in_specs, out_specs=out_specs, check_rep=False
    )
    sharded = jax.jit(smapped, donate_argnums=donate, keep_unused=True)
    _CACHE["sharded_nodonate"] = jax.jit(smapped, keep_unused=True)
    _CACHE["in_names"] = in_names
    _CACHE["out_names"] = out_names
    _CACHE["out_avals"] = out_avals
    _CACHE["mesh"] = mesh

    def run(in_maps, device_only=False):
        concat_in = [
            np.concatenate([np.asarray(in_maps[c][nm]) for c in range(NCORES)], axis=0)
            for nm in in_names
        ]
        concat_zeros = [
            np.zeros((NCORES * av.shape[0], *av.shape[1:]), av.dtype) for av in out_avals
        ]
        out_arrs = sharded(*concat_in, *concat_zeros)
        if device_only:
            for o in out_arrs:
                o.block_until_ready()
            return None
        return [
            {
                nm: np.asarray(out_arrs[i]).reshape(NCORES, *out_avals[i].shape)[c]
                for i, nm in enumerate(out_names)
            }
            for c in range(NCORES)
        ]

    _CACHE["run"] = run
    return run


def _make_in_maps(E_real: np.ndarray, E_imag: np.ndarray):
    offs = _CACHE.get("offs")
    if offs is None:
        offs = _CACHE["offs"] = _build_offsets()
    E_real = np.asarray(E_real, dtype=np.float32)
    E_imag = np.asarray(E_imag, dtype=np.float32)
    in_maps = []
    for c in range(NCORES):
        idx = np.arange(c * WLOC - EHALO, (c + 1) * WLOC + EHALO) % W
        planes = np.empty((8, EW), dtype=NPFP)
        for b in range(B):
            for mu in range(NMODES):
                planes[_pidx(b, mu, 0)] = E_real[b, idx, mu].astype(NPFP)
                planes[_pidx(b, mu, 1)] = E_imag[b, idx, mu].astype(NPFP)
        in_maps.append({"e_planes": planes, "offs": offs})
    return in_maps


def _assemble(results) -> np.ndarray:
    out = np.empty((B, W, NMODES, S), dtype=np.complex64)
    for c in range(NCORES):
        o = results[c]["out"].astype(np.float32).reshape(S, B, 2, 2, WLOC)
        cx = o[:, :, :, 0, :] + 1j * o[:, :, :, 1, :]  # [S, B, mu, WLOC]
        out[:, c * WLOC : (c + 1) * WLOC, :, :] = cx.transpose(1, 3, 2, 0)
    return out


def kernel(E_real: np.ndarray, E_imag: np.ndarray) -> np.ndarray:
    run = _get_runner()
    in_maps = _make_in_maps(E_real, E_imag)
    return _assemble(run(in_maps))


def bench(E_real: np.ndarray, E_imag: np.ndarray, iters: int = 50):
    """Time device execution with device-resident inputs (no donation, no
    host transfers in the loop). Returns (seconds_per_iter, outputs)."""
    import time
    import jax

    run = _get_runner()  # ensures nc built + hook installed
    sharded_nodonate = _CACHE["sharded_nodonate"]
    in_names, out_avals = _CACHE["in_names"], _CACHE["out_avals"]
    in_maps = _make_in_maps(E_real, E_imag)
    concat_in = [
        np.concatenate([np.asarray(in_maps[c][nm]) for c in range(NCORES)], axis=0)
        for nm in in_names
    ]
    concat_zeros = [
        np.zeros((NCORES * av.shape[0], *av.shape[1:]), av.dtype) for av in out_avals
    ]
    from jax.sharding import NamedSharding, PartitionSpec

    shard = NamedSharding(_CACHE["mesh"], PartitionSpec("core"))
    args = [jax.device_put(a, shard) for a in (*concat_in, *concat_zeros)]
    out = sharded_nodonate(*args)  # warm-up/compile
    jax.block_until_ready(out)
    # measure marginal cost per iteration at two batch sizes to cancel
    # fixed per-batch overhead
    def loop(n):
        t0 = time.perf_counter()
        outs = [sharded_nodonate(*args) for _ in range(n)]
        jax.block_until_ready(outs)
        return time.perf_counter() - t0

    loop(3)
    t_small = loop(max(2, iters // 10))
    t_big = loop(iters)
    per_iter = (t_big - t_small) / (iters - max(2, iters // 10))
    return per_iter, out
